# revision 19
# baseline (speedup 1.0000x reference)
"""Beamformer (MoE-style per-frame beam dispatch) for Trainium2, 8 NeuronCores.

Math per frame n (w = W[beam_id[n]]):
    out_r[n,f] = sum_c xr*wr + xi*wi
    out_i[n,f] = sum_c xi*wr - xr*wi          -> out (16384, 2, 257, 1) fp32

Strategy (fp16 data movement, fp32 accumulation):
  * Frames are globally sorted by beam on the host, so each beam occupies one
    contiguous span of the frame axis. The per-frame weight gather then
    becomes a handful of per-beam segments -- no on-device gather at all.
  * Shard the 257 frequency bins: core c owns bins [32c, 32c+32) as 4
    "bingroups" of 8 bins; bin 256 is done on host (1/257 of the work) so the
    SPMD program is identical on all 8 cores. Every core streams all frames.
  * The complex filter-and-sum is a matmul on the tensor engine: contraction
    dim K = 8 bins x 16 (re/im x 8 channels), stationary operand = a
    block-diagonal fp16 weight tile [128, 32] per (beam, bingroup) (16 real
    outputs: 8 bins x re/im), moving operand = transposed fp16 input columns
    (frames). 4 bingroups are packed into the 128 PSUM partitions with
    col-tiled matmuls (tile_position); fp16 operands run the PE at full rate
    (1 row/cycle vs 4 for fp32) while PSUM still accumulates in fp32.
  * Frames stream in variable-size chunks (small head/tail chunks shrink the
    serial pipeline fill/drain). Each chunk: 4 per-bingroup x DMAs [128, F]
    fp16 (finer dependencies -> PE starts after the first one), per
    beam-segment piece (<=512 cols) 4 matmuls -> PSUM, DVE/ACT (alternating)
    copy -> fp16 staging, then 4 compact output DMAs that pull only the 16
    useful rows of each 32-row group.
  * The block-diagonal weight bank is expanded on device from a ~50 KB
    compact fp16 table (Pool memset + 8 tiny DVE copies) instead of DMAing
    the zero-padded 786 KB bank.
  * DMA per core ~19 MB (16.8 in + 2.0 out + 0.1 w), the fp16 roofline
    (~53 us at the 360 GB/s model bandwidth).

Host side: one global transpose/pack of the sorted input (~1 s), per-core
inputs are contiguous slices of it; output is un-permuted at the end.
The Bass program depends only on the beam histogram (segment boundaries are
baked in as static sizes); it is built and compiled on first call.
"""

import numpy as np

NUM_BEAM, NUM_BIN, NUM_CH = 24, 257, 8
N_FRAMES = 16384
NCORES = 8
P = 128
NBIN_DEV = 256                # bins computed on device
NGRP = NBIN_DEV // 8          # 32 bingroups
GPC = NGRP // NCORES          # 4 bingroups per core
NTW = NUM_BEAM * GPC          # 96 weight tiles per core
MAXN = 512                    # max matmul moving dim (one PSUM bank, fp32)

# variable chunk sizes: small head chunk (PE starts early), small tail chunk
CHUNK_SIZES = [1024] + [2048] * 7 + [768, 256]
assert sum(CHUNK_SIZES) == N_FRAMES
CHUNK_BOUNDS = np.concatenate([[0], np.cumsum(CHUNK_SIZES)]).astype(int)
NCH = len(CHUNK_SIZES)

_CACHE = {}
TRACE = False
LAST_RESULTS = None


def _segments(offs):
    """Static per-chunk list of (beam, lo, hi) pieces (local cols, <=MAXN)."""
    chunks = []
    for q in range(NCH):
        n0, n1 = int(CHUNK_BOUNDS[q]), int(CHUNK_BOUNDS[q + 1])
        segs = []
        for b in range(NUM_BEAM):
            s0, s1 = max(offs[b], n0), min(offs[b + 1], n1)
            if s1 <= s0:
                continue
            L = s1 - s0
            npieces = -(-L // MAXN)
            bounds = [s0 + (L * i) // npieces for i in range(npieces + 1)]
            for i in range(npieces):
                segs.append((b, bounds[i] - n0, bounds[i + 1] - n0))
        chunks.append(segs)
    return chunks


def _build_program(offs):
    import concourse.bacc as bacc
    import concourse.bass as bass
    import concourse.tile as tile
    from concourse import mybir

    f16 = mybir.dt.float16
    f32 = mybir.dt.float32
    chunks = _segments(offs)

    nc = bacc.Bacc("TRN2", target_bir_lowering=False, debug=False)
    xt_d = nc.dram_tensor("xt", [GPC, P, N_FRAMES], f16, kind="ExternalInput")
    wt_d = nc.dram_tensor("wt", [P, NTW, 2], f16, kind="ExternalInput")
    out_d = nc.dram_tensor("out", [4 * 16, N_FRAMES], f16, kind="ExternalOutput")

    with tile.TileContext(nc) as tc:
        with (
            tc.tile_pool(name="singles", bufs=1) as singles,
            tc.tile_pool(name="xp", bufs=3) as xp,
            tc.tile_pool(name="st", bufs=3) as stp,
            tc.tile_pool(name="ps", bufs=8, space=bass.MemorySpace.PSUM) as ps,
        ):
            w_cmp = singles.tile([P, NTW, 2], f16)
            w_bank = singles.tile([P, NTW, 32], f16)
            # memset has no input dep: starts at t=0 on the idle Pool engine
            nc.gpsimd.memset(w_bank[:], 0.0)

            ncopy = 0
            w_loaded = False
            for q in range(NCH):
                n0, n1 = int(CHUNK_BOUNDS[q]), int(CHUNK_BOUNDS[q + 1])
                F = n1 - n0
                x_sb = xp.tile([P, GPC, F], f16, tag="x")
                for j in range(GPC):
                    nc.sync.dma_start(
                        out=x_sb[:, j, :], in_=xt_d[j, :, n0:n1]
                    )
                if not w_loaded:
                    # slot the tiny weight DMA right after the first x DMA
                    nc.sync.dma_start(out=w_cmp[:], in_=wt_d[:])
                    # expand compact table into the block-diagonal bank:
                    # w_bank[fs*16+k, t, fs*2+ri'] = w_cmp[fs*16+k, t, ri']
                    for fs in range(8):
                        nc.vector.tensor_copy(
                            w_bank[fs * 16 : (fs + 1) * 16, :, fs * 2 : fs * 2 + 2],
                            w_cmp[fs * 16 : (fs + 1) * 16, :, :],
                        )
                    w_loaded = True

                st = stp.tile([P, F], f16, tag="st")
                for b, lo, hi in chunks[q]:
                    pl = hi - lo
                    acc = ps.tile([P, MAXN], f32, tag="acc")
                    for j in range(GPC):
                        nc.tensor.matmul(
                            acc[32 * j : 32 * j + 32, :pl],
                            w_bank[:, b * GPC + j, :],
                            x_sb[:, j, lo:hi],
                            start=True,
                            stop=True,
                            tile_position=(0, 32 * j),
                        )
                    if ncopy % 2 == 0:
                        nc.vector.tensor_copy(st[:, lo:hi], acc[:, :pl])
                    else:
                        nc.scalar.copy(out=st[:, lo:hi], in_=acc[:, :pl])
                    ncopy += 1
                # compact output: only rows 32j..32j+16 of each group are
                # real -- one DMA with a grouped-partition access pattern,
                # issued from the (otherwise idle) Pool queue so its waits
                # don't head-of-line-block the next x DMA on the SP queue.
                # The last chunks' outs go on SP (cheaper HWDGE gen, and no
                # x DMAs remain to be blocked) to shorten the drain tail.
                if q == NCH - 1:
                    eng = nc.sync       # after the last x DMA; cheap HWDGE gen
                elif q == NCH - 2:
                    eng = nc.scalar     # parallel gen path for the tail
                else:
                    eng = nc.gpsimd
                eng.dma_start(
                    out=out_d[:, n0:n1].rearrange("(g r) n -> g r n", g=GPC),
                    in_=st[:].rearrange("(g r) n -> g r n", g=GPC)[:, 0:16, :],
                )

    nc.compile()
    return nc


def _pack_weights(W):
    """Per-core compact weight tables, each (128, NTW, 2) fp16."""
    wr = W[:, 0]  # (24, 257, 8)
    wi = W[:, 1]
    w16 = np.zeros((NUM_BEAM, NGRP, 8, 16, 2), np.float32)  # b, g, fs, k, ri'
    for g in range(NGRP):
        for fs in range(8):
            fb = g * 8 + fs
            w16[:, g, fs, 0:8, 0] = wr[:, fb]
            w16[:, g, fs, 8:16, 0] = wi[:, fb]
            w16[:, g, fs, 0:8, 1] = -wi[:, fb]
            w16[:, g, fs, 8:16, 1] = wr[:, fb]
    out = []
    for c in range(NCORES):
        sl = w16[:, c * GPC : (c + 1) * GPC]  # (24, GPC, 8, 16, 2)
        out.append(
            np.ascontiguousarray(
                sl.transpose(2, 3, 0, 1, 4).reshape(P, NTW, 2).astype(np.float16)
            )
        )
    return out


def _pack_x_global(inp, perm):
    """x_t (NGRP, 128, N) fp16: [g, fs*16+ri*8+c, n] = inp[perm[n], ri, 8g+fs, c]."""
    xs = inp[perm][:, :, :NBIN_DEV, :]  # (N, 2, 256, 8)
    arr = xs.reshape(N_FRAMES, 2, NGRP, 8, NUM_CH).transpose(2, 3, 1, 4, 0)
    return np.ascontiguousarray(arr.reshape(NGRP, P, N_FRAMES).astype(np.float16))


def kernel(**inputs):
    global LAST_RESULTS
    from concourse.bass_utils import run_bass_kernel_spmd

    inp = np.ascontiguousarray(np.asarray(inputs["input"], dtype=np.float32))
    W = np.ascontiguousarray(np.asarray(inputs["W"], dtype=np.float32))
    bid = np.asarray(inputs["beam_id"]).astype(np.int64)

    perm = np.argsort(bid, kind="stable")
    counts = np.bincount(bid, minlength=NUM_BEAM)
    offs = np.concatenate([[0], np.cumsum(counts)]).astype(int)

    key = tuple(offs)
    if key not in _CACHE:
        _CACHE[key] = _build_program(offs)
    nc = _CACHE[key]

    wts = _pack_weights(W)
    xt = _pack_x_global(inp, perm)
    in_maps = [
        {"xt": xt[c * GPC : (c + 1) * GPC], "wt": wts[c]} for c in range(NCORES)
    ]

    res = run_bass_kernel_spmd(nc, in_maps, list(range(NCORES)), trace=TRACE)
    LAST_RESULTS = res

    # device row 16j+m holds (bingroup j, m = fs*2+ri)
    out_sorted = np.empty((N_FRAMES, 2, NUM_BIN), np.float32)
    for c in range(NCORES):
        ot = np.asarray(res.results[c]["out"], dtype=np.float32)  # (64, N)
        a = ot.reshape(GPC, 8, 2, N_FRAMES).transpose(3, 2, 0, 1)
        out_sorted[:, :, 32 * c : 32 * c + 32] = a.reshape(N_FRAMES, 2, 32)

    # bin 256 on host (keeps the device bin count divisible by 8 cores)
    xs = inp[:, :, NUM_BIN - 1, :]
    ws = W[bid][:, :, NUM_BIN - 1, :]
    xr, xi = xs[:, 0], xs[:, 1]
    wr, wi = ws[:, 0], ws[:, 1]

    out_full = np.empty((N_FRAMES, 2, NUM_BIN), np.float32)
    out_full[perm] = out_sorted
    out_full[:, 0, NUM_BIN - 1] = (xr * wr + xi * wi).sum(-1)
    out_full[:, 1, NUM_BIN - 1] = (xi * wr - xr * wi).sum(-1)
    return out_full.reshape(N_FRAMES, 2, NUM_BIN, 1)


# revision 20
# speedup vs baseline: 1.0472x; 1.0472x over previous
"""Beamformer (MoE-style per-frame beam dispatch) for Trainium2, 8 NeuronCores.

Math per frame n (w = W[beam_id[n]]):
    out_r[n,f] = sum_c xr*wr + xi*wi
    out_i[n,f] = sum_c xi*wr - xr*wi          -> out (16384, 2, 257, 1) fp32

Strategy (fp16 data movement, fp32 accumulation):
  * Frames are globally sorted by beam on the host, so each beam occupies one
    contiguous span of the frame axis. The per-frame weight gather then
    becomes a handful of per-beam segments -- no on-device gather at all.
  * Shard the 257 frequency bins: core c owns bins [32c, 32c+32) as 4
    "bingroups" of 8 bins; bin 256 is done on host (1/257 of the work) so the
    SPMD program is identical on all 8 cores. Every core streams all frames.
  * The complex filter-and-sum is a matmul on the tensor engine: contraction
    dim K = 8 bins x 16 (re/im x 8 channels), stationary operand = a
    block-diagonal fp16 weight tile [128, 32] per (beam, bingroup) (16 real
    outputs: 8 bins x re/im), moving operand = transposed fp16 input columns
    (frames). 4 bingroups are packed into the 128 PSUM partitions with
    col-tiled matmuls (tile_position); fp16 operands run the PE at full rate
    (1 row/cycle vs 4 for fp32) while PSUM still accumulates in fp32.
  * Frames stream in variable-size chunks (small head/tail chunks shrink the
    serial pipeline fill/drain). Each chunk: 4 per-bingroup x DMAs [128, F]
    fp16 (finer dependencies -> PE starts after the first one), per
    beam-segment piece (<=512 cols) 4 matmuls -> PSUM, DVE/ACT (alternating)
    copy -> fp16 staging, then 4 compact output DMAs that pull only the 16
    useful rows of each 32-row group.
  * The block-diagonal weight bank is expanded on device from a ~50 KB
    compact fp16 table (Pool memset + 8 tiny DVE copies) instead of DMAing
    the zero-padded 786 KB bank.
  * DMA per core ~19 MB (16.8 in + 2.0 out + 0.1 w), the fp16 roofline
    (~53 us at the 360 GB/s model bandwidth).

Host side: one global transpose/pack of the sorted input (~1 s), per-core
inputs are contiguous slices of it; output is un-permuted at the end.
The Bass program depends only on the beam histogram (segment boundaries are
baked in as static sizes); it is built and compiled on first call.
"""

import numpy as np

NUM_BEAM, NUM_BIN, NUM_CH = 24, 257, 8
N_FRAMES = 16384
NCORES = 8
P = 128
NBIN_DEV = 256                # bins computed on device
NGRP = NBIN_DEV // 8          # 32 bingroups
GPC = NGRP // NCORES          # 4 bingroups per core
NTW = NUM_BEAM * GPC          # 96 weight tiles per core
MAXN = 512                    # max matmul moving dim (one PSUM bank, fp32)

# variable chunk sizes: small head chunk (PE starts early), small tail chunk
CHUNK_SIZES = [1024] + [2048] * 7 + [768, 256]
assert sum(CHUNK_SIZES) == N_FRAMES
CHUNK_BOUNDS = np.concatenate([[0], np.cumsum(CHUNK_SIZES)]).astype(int)
NCH = len(CHUNK_SIZES)

_CACHE = {}
TRACE = False
LAST_RESULTS = None


def _segments(offs):
    """Static per-chunk list of (beam, lo, hi) pieces (local cols, <=MAXN)."""
    chunks = []
    for q in range(NCH):
        n0, n1 = int(CHUNK_BOUNDS[q]), int(CHUNK_BOUNDS[q + 1])
        segs = []
        for b in range(NUM_BEAM):
            s0, s1 = max(offs[b], n0), min(offs[b + 1], n1)
            if s1 <= s0:
                continue
            L = s1 - s0
            npieces = -(-L // MAXN)
            bounds = [s0 + (L * i) // npieces for i in range(npieces + 1)]
            for i in range(npieces):
                segs.append((b, bounds[i] - n0, bounds[i + 1] - n0))
        chunks.append(segs)
    return chunks


def _build_program(offs):
    import concourse.bacc as bacc
    import concourse.bass as bass
    import concourse.tile as tile
    from concourse import mybir

    f16 = mybir.dt.float16
    f32 = mybir.dt.float32
    chunks = _segments(offs)

    nc = bacc.Bacc("TRN2", target_bir_lowering=False, debug=False)
    xt_d = nc.dram_tensor("xt", [GPC, P, N_FRAMES], f16, kind="ExternalInput")
    wt_d = nc.dram_tensor("wt", [P, NTW, 2], f16, kind="ExternalInput")
    out_d = nc.dram_tensor("out", [4 * 16, N_FRAMES], f16, kind="ExternalOutput")

    with tile.TileContext(nc) as tc:
        with (
            tc.tile_pool(name="singles", bufs=1) as singles,
            tc.tile_pool(name="xp", bufs=3) as xp,
            tc.tile_pool(name="st", bufs=3) as stp,
            tc.tile_pool(name="ps", bufs=8, space=bass.MemorySpace.PSUM) as ps,
        ):
            w_cmp = singles.tile([P, NTW, 2], f16)
            w_bank = singles.tile([P, NTW, 32], f16)
            # memset has no input dep: starts at t=0 on the idle Pool engine
            nc.gpsimd.memset(w_bank[:], 0.0)

            ncopy = 0
            w_loaded = False
            for q in range(NCH):
                n0, n1 = int(CHUNK_BOUNDS[q]), int(CHUNK_BOUNDS[q + 1])
                F = n1 - n0
                x_sb = xp.tile([P, GPC, F], f16, tag="x")
                for j in range(GPC):
                    nc.sync.dma_start(
                        out=x_sb[:, j, :], in_=xt_d[j, :, n0:n1]
                    )
                if not w_loaded:
                    # slot the tiny weight DMA right after the first x DMA
                    nc.sync.dma_start(out=w_cmp[:], in_=wt_d[:])
                    # expand compact table into the block-diagonal bank:
                    # w_bank[fs*16+k, t, fs*2+ri'] = w_cmp[fs*16+k, t, ri']
                    for fs in range(8):
                        nc.vector.tensor_copy(
                            w_bank[fs * 16 : (fs + 1) * 16, :, fs * 2 : fs * 2 + 2],
                            w_cmp[fs * 16 : (fs + 1) * 16, :, :],
                        )
                    w_loaded = True

                st = stp.tile([P, F], f16, tag="st")
                for b, lo, hi in chunks[q]:
                    pl = hi - lo
                    acc = ps.tile([P, MAXN], f32, tag="acc")
                    for j in range(GPC):
                        nc.tensor.matmul(
                            acc[32 * j : 32 * j + 32, :pl],
                            w_bank[:, b * GPC + j, :],
                            x_sb[:, j, lo:hi],
                            start=True,
                            stop=True,
                            tile_position=(0, 32 * j),
                        )
                    if ncopy % 2 == 0:
                        nc.vector.tensor_copy(st[:, lo:hi], acc[:, :pl])
                    else:
                        nc.scalar.copy(out=st[:, lo:hi], in_=acc[:, :pl])
                    ncopy += 1
                # compact output: only rows 32j..32j+16 of each group are
                # real -- one DMA with a grouped-partition access pattern,
                # issued from the (otherwise idle) Pool queue so its waits
                # don't head-of-line-block the next x DMA on the SP queue.
                # The last chunks' outs go on SP (cheaper HWDGE gen, and no
                # x DMAs remain to be blocked) to shorten the drain tail.
                eng = nc.sync if q == NCH - 1 else nc.gpsimd
                eng.dma_start(
                    out=out_d[:, n0:n1].rearrange("(g r) n -> g r n", g=GPC),
                    in_=st[:].rearrange("(g r) n -> g r n", g=GPC)[:, 0:16, :],
                )

    nc.compile()
    return nc


def _pack_weights(W):
    """Per-core compact weight tables, each (128, NTW, 2) fp16."""
    wr = W[:, 0]  # (24, 257, 8)
    wi = W[:, 1]
    w16 = np.zeros((NUM_BEAM, NGRP, 8, 16, 2), np.float32)  # b, g, fs, k, ri'
    for g in range(NGRP):
        for fs in range(8):
            fb = g * 8 + fs
            w16[:, g, fs, 0:8, 0] = wr[:, fb]
            w16[:, g, fs, 8:16, 0] = wi[:, fb]
            w16[:, g, fs, 0:8, 1] = -wi[:, fb]
            w16[:, g, fs, 8:16, 1] = wr[:, fb]
    out = []
    for c in range(NCORES):
        sl = w16[:, c * GPC : (c + 1) * GPC]  # (24, GPC, 8, 16, 2)
        out.append(
            np.ascontiguousarray(
                sl.transpose(2, 3, 0, 1, 4).reshape(P, NTW, 2).astype(np.float16)
            )
        )
    return out


def _pack_x_global(inp, perm):
    """x_t (NGRP, 128, N) fp16: [g, fs*16+ri*8+c, n] = inp[perm[n], ri, 8g+fs, c]."""
    xs = inp[perm][:, :, :NBIN_DEV, :]  # (N, 2, 256, 8)
    arr = xs.reshape(N_FRAMES, 2, NGRP, 8, NUM_CH).transpose(2, 3, 1, 4, 0)
    return np.ascontiguousarray(arr.reshape(NGRP, P, N_FRAMES).astype(np.float16))


def kernel(**inputs):
    global LAST_RESULTS
    from concourse.bass_utils import run_bass_kernel_spmd

    inp = np.ascontiguousarray(np.asarray(inputs["input"], dtype=np.float32))
    W = np.ascontiguousarray(np.asarray(inputs["W"], dtype=np.float32))
    bid = np.asarray(inputs["beam_id"]).astype(np.int64)

    perm = np.argsort(bid, kind="stable")
    counts = np.bincount(bid, minlength=NUM_BEAM)
    offs = np.concatenate([[0], np.cumsum(counts)]).astype(int)

    key = tuple(offs)
    if key not in _CACHE:
        _CACHE[key] = _build_program(offs)
    nc = _CACHE[key]

    wts = _pack_weights(W)
    xt = _pack_x_global(inp, perm)
    in_maps = [
        {"xt": xt[c * GPC : (c + 1) * GPC], "wt": wts[c]} for c in range(NCORES)
    ]

    res = run_bass_kernel_spmd(nc, in_maps, list(range(NCORES)), trace=TRACE)
    LAST_RESULTS = res

    # device row 16j+m holds (bingroup j, m = fs*2+ri)
    out_sorted = np.empty((N_FRAMES, 2, NUM_BIN), np.float32)
    for c in range(NCORES):
        ot = np.asarray(res.results[c]["out"], dtype=np.float32)  # (64, N)
        a = ot.reshape(GPC, 8, 2, N_FRAMES).transpose(3, 2, 0, 1)
        out_sorted[:, :, 32 * c : 32 * c + 32] = a.reshape(N_FRAMES, 2, 32)

    # bin 256 on host (keeps the device bin count divisible by 8 cores)
    xs = inp[:, :, NUM_BIN - 1, :]
    ws = W[bid][:, :, NUM_BIN - 1, :]
    xr, xi = xs[:, 0], xs[:, 1]
    wr, wi = ws[:, 0], ws[:, 1]

    out_full = np.empty((N_FRAMES, 2, NUM_BIN), np.float32)
    out_full[perm] = out_sorted
    out_full[:, 0, NUM_BIN - 1] = (xr * wr + xi * wi).sum(-1)
    out_full[:, 1, NUM_BIN - 1] = (xi * wr - xr * wi).sum(-1)
    return out_full.reshape(N_FRAMES, 2, NUM_BIN, 1)
